# revision 65
# baseline (speedup 1.0000x reference)
# Trainium2 Bass kernel for nn_LocalCrossAttentionModule.
#
# Math: softmax over a size-1 axis is identically 1, so q/k (and x_query,
# Wq, bq, Wk, bk) never affect the output. The module reduces to, per
# 5x5 patch p (576 of them = 4 batch x 12x12 grid, stride 36):
#   kvf_p  = flatten(x_kv patch)                  (3200,)
#   v_p    = Wv @ kvf_p + bv                      (1600,) viewed as (64, 5, 5)
#   z_p    = conv_w @ v_p[:, s] + conv_b          (128,) per pixel s in 5x5
# z_p is scattered into an otherwise-constant (conv_b) output map.
#
# Sharding (8 cores = F4 x P2): F = 6 whole pixels of the pixel-major
# weight matrix (384 rows) + a 32-row slice of W2 = conv_w @ Wv[pixel24]
# (folding pixel 24's conv through mm1); P = patch half (288). No
# contraction split: the serial psum->SBUF->HBM tail is what limits this
# kernel, so output volume per core is kept minimal (single final z, no
# partials). All biases are added on the host (linear, added exactly once).
#
# Device stream: ONE packed DRAM tensor per core, [128, 25, 416+288] f16
# = per k-tile slot [lhsT block row | kv rhs]; per-partition slot runs are
# contiguous (1408B descriptors). Each HWDGE dma_start costs ~625ns of
# issue occupancy on its engine, so the stream is ~10 chunks alternated
# between the two HWDGE rings (sync + scalar). The PE starts at 50% HAM
# duty (240ns/matmul) and is granted full duty (122ns) only after ~4.5us
# of gap-free activity, so warm-up matmuls + early fillers keep it busy
# from the instant the engine preamble ends.

import numpy as np

B = 4
CKV = 128
HW_ = 432
E = 2
PP = 5           # patch side
STRIDE = 36
PI = 12          # patch grid side
NP = B * PI * PI      # 576 patches
KF = CKV * PP * PP    # 3200 kv features per patch
OUT = 64
O2 = 128
NCORES = 8

KTS = 25         # k-tile slots (full contraction)
NPIX = 6         # whole pixels per feature shard
RW = NPIX * OUT + 32  # 416 lhsT cols: 384 pixel rows + 32 folded W2 rows
NBLK = 4         # lhsT blocks per slot (3 full 128 + one 32-wide)
NCH = 288        # patches per core (P half)
SLOT = RW + NCH  # packed f16 cols per slot: [w 416 | kv 288]
# (slots, engine): engine 0=sync HWDGE, 1=scalar HWDGE, 2=gpsimd SWDGE.
# Alternating 3-slot chunks measured best on the HW rings (the 8-core
# stream runs at the chip HBM roofline; finer chunks add sem overhead,
# coarser ones stall the PE). The 4 TAIL slots ride the otherwise-idle
# SWDGE ring as one early DMA: they land by ~13us instead of ~21us, so
# mm1's end is PE-bound instead of supply-bound, and the HW rings carry
# 16% fewer bytes.
# (start, slots, engine) — issued in list order. A tail chunk on the
# gpsimd SWDGE ring was tried and REGRESSED ~8us: the SW ring drains
# immediately and steals early HBM bandwidth from the head chunks,
# delaying the HAM grant. Keep everything on the two HWDGE rings.
# A leaner 1/2-slot head was tried and regressed ~3us (repeated PE
# starvation: small chunks pay the ~0.9us completion latency per chunk
# with little data); the 3-slot head + 12 warms is the measured optimum.
CHUNKS = [(0, 3, 0), (3, 3, 1), (6, 3, 0), (9, 3, 1), (12, 3, 0),
          (15, 3, 1), (18, 3, 0), (21, 1, 1), (22, 1, 0), (23, 1, 1),
          (24, 1, 0)]
N_WARM = 12      # warm-up matmuls: hold PE activity until first chunk lands

_PROGRAM = {}


def _build_program():
    import concourse.mybir as mybir
    from concourse import bacc
    from concourse.tile import TileContext

    f32 = mybir.dt.float32
    f16 = mybir.dt.float16

    nc = bacc.Bacc()
    wk_d = nc.declare_dram_parameter("wk", [128, KTS, SLOT], f16, isOutput=False)
    cw_d = nc.declare_dram_parameter("cw", [128, 256], f16, isOutput=False)
    z_d = nc.declare_dram_parameter("z", [128, NPIX, NCH], f16, isOutput=True)
    z24_d = nc.declare_dram_parameter("z24", [32, NCH], f16, isOutput=True)

    with TileContext(nc) as tc:
        with (
            tc.tile_pool(name="consts", bufs=1) as cpool,
            tc.tile_pool(name="wbig", bufs=1) as wpool,
            tc.tile_pool(name="vbuf", bufs=1) as vpool,
            tc.tile_pool(name="zbuf", bufs=1) as zpool,
            tc.tile_pool(name="ps", bufs=1, space="PSUM") as ps,
        ):
            # 8 PSUM banks: wps = warm/filler, psv0-3 = mm1 accumulators
            # (psv3 rows 0:32 = folded pixel-24 output), psz0-2 = mm2 ring.
            # (Bank-PAIR padded tiles with merged 2-bank extracts were
            # tried: each extract op got cheaper (758 vs 894ns/pair) but
            # the coarser store/WAR granularity ate the gain — net ~-0.8us.)
            wps = ps.tile([128, NCH], f32, name="wps")
            ps_v = [ps.tile([128, NCH], f32, name=f"psv{m}") for m in range(NBLK)]
            ps_z = [ps.tile([128, NCH], f32, name=f"psz{j}") for j in range(3)]

            # warm_t MUST be fully zeroed, and not only for the Tile
            # allocator: the HAM duty governor is power-based, and warm
            # matmuls on garbage data (random NaN/denormal f16, high
            # toggle rates) hold the 50%-duty limit ~3us longer than
            # all-zero warm-ups (measured: grant at 13.9us vs 10.5-11.5us).
            warm_t = cpool.tile([128, NCH], f16, name="warm_t")
            nc.vector.memset(warm_t[:], 0.0)
            cw_t = cpool.tile([128, 256], f16, name="cw_t")

            # warm-up: dependency-only-on-memset zero matmuls keep the PE
            # busy from preamble exit so the HAM full-duty grant (~4.5us
            # of gap-free activity) arrives as early as possible
            for _ in range(N_WARM):
                nc.tensor.matmul(
                    wps[:], lhsT=warm_t[:, 0:128], rhs=warm_t[:],
                    start=True, stop=True,
                )

            # input stream spread over three DMA rings
            wk_t = wpool.tile([128, KTS, SLOT], f16, name="wk_t")
            engs = [nc.sync, nc.scalar, nc.gpsimd]
            for ci, (lo, sz, ei) in enumerate(CHUNKS):
                engs[ei].dma_start(wk_t[:, lo:lo + sz, :], wk_d[:, lo:lo + sz, :])
                if ci == 5:
                    nc.scalar.dma_start(cw_t[:], cw_d[:])

            # mm1: V[r, n] = sum_j A[r, j] * KVF[j, n] over 25 slots.
            # Every lhsT is FULL 128-wide: a narrow (32-col) last block
            # makes the PE switch into column-group mode, which flushes
            # the LDWEIGHTS/MATMUL pipeline (~+400ns per slot). Block 3
            # simply spills into the kv region as junk weights; its junk
            # psum rows 32:127 are never read.
            # Last three slots run BANK-MAJOR (m outer) so bank m closes
            # 3*(3-m) matmuls before mm1's end — the v extracts then hide
            # under mm1's tail instead of serializing after it.
            def mm1_order():
                for k in range(KTS - 3):
                    for m in range(NBLK):
                        yield k, m
                for m in range(NBLK):
                    for k in range(KTS - 3, KTS):
                        yield k, m

            for k, m in mm1_order():
                nc.tensor.matmul(
                    ps_v[m][:],
                    lhsT=wk_t[:, k, m * 128:(m + 1) * 128],
                    rhs=wk_t[:, k, RW:SLOT],
                    start=(k == 0),
                    stop=(k == KTS - 1),
                )
                if m == NBLK - 1 and (k < 4 or (k % 3 == 0 and k < 22)):
                    # keep-warm filler: bridges early DMA-supply stalls
                    # while the full-duty grant is pending, and marks PE
                    # activity during mid-stream stalls — a ~2us+ gap
                    # REVOKES the full-duty grant (seen as k=8 -> k=4 HAM
                    # transitions costing ~4us)
                    nc.tensor.matmul(
                        wps[:, 0:128], lhsT=warm_t[:, 0:128],
                        rhs=warm_t[:, 0:128], start=True, stop=True,
                    )

            # V to SBUF as f16 (no bias: host adds all biases once).
            # Pair-packed: bank m = pixels (2m, 2m+1) in partition halves.
            # Extracts split across DVE and ACT (gpsimd has no PSUM port).
            v_t = []
            for m in range(3):
                vt = vpool.tile([128, NCH], f16, name=f"vt{m}")
                if m == 1:
                    nc.scalar.copy(vt[:], ps_v[m][:])
                else:
                    nc.vector.tensor_scalar_add(vt[:], ps_v[m][:], 0.0)
                v_t.append(vt)
            # z24 store on sync (idle here): keeping it off gpsimd leaves
            # that ring a single clean z-slab store — hot-device traces
            # showed gpsimd serializing z24 + slab with ~0.7us idle gaps
            z24_t = zpool.tile([32, NCH], f16, name="z24_t")
            nc.vector.tensor_scalar_add(z24_t[:], ps_v[3][0:32, :], 0.0)
            nc.sync.dma_start(z24_d[:], z24_t[:])

            # mm2: z[o2, n] = conv_w @ v[:, s]; masked cw variant h selects
            # the pixel in partition half h. Extracts alternate DVE/ACT;
            # mm2 uses the 3 dedicated psz banks plus wps and the freed
            # psv0/psv1 banks, so no mm2 matmul waits on a z-extract WAR.
            z_banks = ps_z + [wps, ps_v[0], ps_v[1]]
            z_t = zpool.tile([128, NPIX, NCH], f16, name="z_t")
            for s in range(NPIX):
                m, h = divmod(s, 2)
                psz = z_banks[s]
                nc.tensor.matmul(
                    psz[:],
                    lhsT=cw_t[:, 128 * h:128 * (h + 1)],
                    rhs=v_t[m][:],
                    start=True, stop=True,
                )
                if s % 2 == 0:
                    nc.vector.tensor_scalar_add(z_t[:, s, :], psz[:], 0.0)
                else:
                    nc.scalar.copy(z_t[:, s, :], psz[:])
                # eager stores on three rings. The FINAL slab rides the
                # scalar ring: scalar itself produced the z5 extract, so
                # the store issue is self-gated (no cross-engine semaphore
                # hop), its ring is drained, and it has no wake-up lag —
                # gpsimd showed ~1.1us of idle-wake latency before its
                # store issue in traces.
                if s in (1, 3, 5):
                    a = {1: 0, 3: 2, 5: 4}[s]
                    eng = {1: nc.sync, 3: nc.gpsimd, 5: nc.scalar}[s]
                    eng.dma_start(z_d[:, a:s + 1, :], z_t[:, a:s + 1, :])
    nc.finalize()
    return nc


def _get_program():
    if "p" not in _PROGRAM:
        _PROGRAM["p"] = _build_program()
    return _PROGRAM["p"]


def _prep_in_maps(x_kv, Wv, conv_w):
    """Host-side shard/layout prep. Returns list of per-core input dicts."""
    x_kv = np.ascontiguousarray(np.asarray(x_kv, dtype=np.float32))
    Wv = np.asarray(Wv, dtype=np.float32)
    conv_w = np.asarray(conv_w, dtype=np.float32)

    # gather all 5x5 patches (padded coords: top-left of patch (pi,pj) is
    # original coords (pi*36-2, pj*36-2))
    pad = np.zeros((B, CKV, HW_ + 2 * E, HW_ + 2 * E), np.float32)
    pad[:, :, E:HW_ + E, E:HW_ + E] = x_kv
    r = (np.arange(PI)[:, None] * STRIDE + np.arange(PP)).ravel()  # (60,)
    g = pad[:, :, r[:, None], r[None, :]]                # (B, C, 60, 60)
    g = g.reshape(B, CKV, PI, PP, PI, PP)
    # feature j = c*25 + pr*5 + pc ; patch n = b*144 + pi*12 + pj
    kvf_t = g.transpose(1, 3, 5, 0, 2, 4).reshape(KF, NP)     # (3200, 576)
    kv_arr = kvf_t.reshape(KTS, 128, NP).transpose(1, 0, 2)   # (128, 25, 576)

    # conv folded into the 25th pixel's weights
    perm24 = np.array([o * PP * PP + 24 for o in range(OUT)], np.int64)
    W2 = conv_w @ Wv[perm24]                 # (128, 3200)

    cw = np.zeros((128, 256), np.float32)
    cw[0:OUT, 0:128] = conv_w.T
    cw[OUT:128, 128:256] = conv_w.T
    cw = np.ascontiguousarray(cw).astype(np.float16)

    in_maps = [None] * NCORES
    for f in range(4):
        pixels = range(NPIX * f, NPIX * (f + 1))
        perm = np.array(
            [o * PP * PP + s for s in pixels for o in range(OUT)], np.int64
        )  # 384, layout col = s_local*64 + o
        A = np.concatenate(
            [Wv[perm], W2[32 * f:32 * (f + 1)]], axis=0
        )  # (416, 3200): 384 pixel rows + 32 folded rows
        w_arr = A.T.reshape(KTS, 128, RW).transpose(1, 0, 2)  # (128, 25, 416)
        for p in range(2):
            packed = np.concatenate(
                [w_arr, kv_arr[:, :, NCH * p:NCH * (p + 1)]], axis=2
            )  # (128, 25, 704)
            in_maps[2 * f + p] = {
                "wk": np.ascontiguousarray(packed).astype(np.float16),
                "cw": cw,
            }
    return in_maps


def _assemble(results, bv, conv_w, conv_b, out_dtype=np.float32):
    """Add biases once and scatter z into the full (B, 128, 432, 432) map."""
    bv = np.asarray(bv, dtype=np.float32)
    conv_w = np.asarray(conv_w, dtype=np.float32)
    conv_b = np.asarray(conv_b, dtype=np.float32)

    # Bias[o2, s] = conv_w @ bv[pixel s rows] + conv_b
    bias = conv_w @ bv.reshape(OUT, PP * PP) + conv_b[:, None]  # (128, 25)

    y = np.empty((B, O2, HW_, HW_), np.float32)
    y[:] = conv_b.reshape(1, O2, 1, 1)
    base = np.arange(PI) * STRIDE
    for f in range(4):
        for p in range(2):
            c = 2 * f + p
            bs = slice(2 * p, 2 * p + 2)
            z = np.asarray(results[c]["z"], np.float32)      # (128, 6, 288)
            for si in range(NPIX):
                s = NPIX * f + si
                pr, pc = divmod(s, PP)
                blk = z[:, si, :] + bias[:, s:s + 1]
                blk = blk.reshape(O2, 2, PI, PI).transpose(1, 0, 2, 3)
                y[bs, :, (base + pr)[:, None], (base + pc)[None, :]] = blk
            z24 = np.asarray(results[c]["z24"], np.float32)  # (32, 288)
            z24 = z24 + bias[32 * f:32 * (f + 1), 24:25]
            blk = z24.reshape(32, 2, PI, PI).transpose(1, 0, 2, 3)
            y[bs, 32 * f:32 * (f + 1),
              (base + PP - 1)[:, None], (base + PP - 1)[None, :]] = blk
    return y.astype(out_dtype, copy=False)


def _run(inputs, trace=False, trace_kwargs=None):
    from concourse.bass_utils import run_bass_kernel_spmd

    in_maps = _prep_in_maps(inputs["x_kv"], inputs["Wv"], inputs["conv_w"])
    nc = _get_program()
    kw = {}
    if trace:
        kw["trace"] = True
        if trace_kwargs:
            kw.update(trace_kwargs)
    res = run_bass_kernel_spmd(nc, in_maps, list(range(NCORES)), **kw)
    out = _assemble(
        res.results, inputs["bv"], inputs["conv_w"], inputs["conv_b"]
    )
    return out, res


def kernel(**inputs):
    out, _ = _run(inputs, trace=False)
    return out
